# revision 2
# baseline (speedup 1.0000x reference)
"""Spiking transformer block (nn_Block_22170621182450) on 8 trn2 NeuronCores. v2.

Data-parallel over B (2 batch elems/core), channel-major [C_out, tokens]
on-chip layout, tokens t-major. BN stats globalized with one collective per
stage (4 total: qkv-batched, p, f1, f2) + local deterministic reduce.

Precision (flip-sim validated, rel ~1.4e-2 vs the 2e-2 gate): ALL matmuls
single-pass f32r (hw rounds operands to 11 mantissa bits, accumulates
fp32 - probed) except attention (bf16, exact on binary spikes / small
integers). Residual uses x_hi (drops x_lo: ~1e-4 rel, negligible).

Structure: per-timestep software pipeline qkv-LIF -> attention -> o-LIF ->
p-matmul; p-LIF fuses spike+residual into xr; f1 matmul per-ts behind
p-LIF; f1-LIF tile-wise with in-place f32r spikes feeding f2 accumulation.
h_p, xr, h_f2 round-trip through DRAM to fit SBUF (pool stack is LIFO).
"""

import os
import sys

for p in ("/opt/trn_rl_repo", "/root/.axon_site", "/root/.axon_site/_ro/trn_rl_repo",
          "/root/.axon_site/_ro/pypackages"):
    if os.path.isdir(p) and p not in sys.path:
        sys.path.append(p)

import numpy as np

from contextlib import ExitStack
import concourse.bass as bass
import concourse.bacc as bacc
import concourse.tile as tile
from concourse import mybir
from concourse import bass_utils
from concourse import dve_ops as _dve_ops
from concourse.dve_ops import TENSOR_MASK
from concourse.dve_spec import (Spec as _Spec, Src0 as _Src0, Src1 as _Src1,
                                C0 as _C0, C1 as _C1, C2 as _C2,
                                Zero as _Zero, select as _select,
                                lower as _dve_lower,
                                _has_src1 as _has_src1)
from concourse.dve_uop import DveOpSpec as _DveOpSpec
from concourse.masks import make_identity


def _register_lif_op():
    """Fused LIF membrane update: out = (h*a + c) + mask(m < thr ? m : 0).
    One DVE op instead of affine_then_add + TENSOR_MASK."""
    for o in _dve_ops.OPS:
        if o.name == "LIF_UPD_ANT":
            return o
    spec = _Spec(
        body=(_Src0 * _C0 + _C1) + _select(_Src1 < _C2, _Src1, _Zero),
        reference=lambda in0, in1, s0, s1, imm2: (
            (in0.astype(np.float32) * s0 + s1)
            + np.where(in1 < imm2, in1, 0.0)).astype(np.float32),
    )
    row = max(_dve_ops._SUB_OPCODE_FOR_NAME.values()) + 1
    assert row < 0x20
    _dve_ops._SUB_OPCODE_FOR_NAME["LIF_UPD_ANT"] = row
    shas = {}
    for ver in ("v3", "v4"):
        try:
            sp = _DveOpSpec(name="LIF_UPD_ANT", opcode=row,
                            uops=_dve_lower(spec, ver=ver),
                            rd1_en=_has_src1(spec))
            shas[ver] = sp.sha(ver)
        except Exception:
            pass
    op = _dve_ops.DveOp("LIF_UPD_ANT", spec, subdim=False, uops_sha=shas)
    _dve_ops.OPS.append(op)
    _dve_ops.CUSTOM_DVE_SPECS["LIF_UPD_ANT"] = spec
    return op


LIF_UPD = _register_lif_op()

F32 = mybir.dt.float32
F32R = mybir.dt.float32r
BF16 = mybir.dt.bfloat16
AX = mybir.AluOpType
AF = mybir.ActivationFunctionType

T, B, N, C, H = 4, 16, 256, 512, 8
HID = 2048
NCORES = 8
BL = B // NCORES            # 2 batch elems per core
S = T * BL * N              # 2048 tokens per core
SLAB = BL * N               # 512 tokens per time step
S_TOT = T * B * N           # 16384 tokens globally (BN population)
CT_C = C // 128             # 4 channel tiles for C
CT_H = HID // 128           # 16 channel tiles for HID
EPS = 1e-5
SCALE = 0.125
P = 128

_CACHE = {}
USE_AG = os.environ.get("KERNEL_AG", "1") == "1"
FUSED_LIF = os.environ.get("KERNEL_FUSED_LIF", "1") == "1"
PHASES = int(os.environ.get("KERNEL_PHASES", "5"))
USE_TTR = os.environ.get("KERNEL_TTR", "1") == "1"
USE_STT = os.environ.get("KERNEL_STT", "1") == "1"
USE_POOL = os.environ.get("KERNEL_POOL", "0") == "1"
INTERLEAVE_F2 = os.environ.get("KERNEL_INTERLEAVE_F2", "1") == "1"
NO_COLL = os.environ.get("KERNEL_NO_COLL", "0") == "1"
RG = [list(range(NCORES))]


def _round_mant(x, m=11):
    """Round fp32 to m explicit mantissa bits (float32r grid)."""
    x = np.ascontiguousarray(x, np.float32)
    b = x.view(np.uint32).astype(np.uint64)
    shift = 23 - m
    add = np.uint64(1 << (shift - 1))
    mask = np.uint64(~((1 << shift) - 1) & 0xFFFFFFFF)
    return ((b + add) & mask).astype(np.uint32).view(np.float32)


def _pack_ch(v, n_ct):
    """[n_ct*128] channel vector -> [128, n_ct] (channel%128 on partitions)."""
    return np.ascontiguousarray(np.asarray(v, np.float32).reshape(n_ct, P).T)


class _PhaseStop(Exception):
    pass


def _build():
    nc = bacc.Bacc("TRN2", target_bir_lowering=False, debug=False,
                   num_devices=NCORES)

    def dt_in(name, shape, dtype):
        return nc.dram_tensor(name, list(shape), dtype,
                              kind="ExternalInput").ap()

    xt = dt_in("xt", (C, S), F32R)            # x_hi, channel-major
    w_lin = {nm: dt_in(f"w_{nm}", (C, C), F32R) for nm in ("q", "k", "v", "p")}
    w_f1 = dt_in("w_f1", (C, HID), F32R)
    w_f2 = dt_in("w_f2", (HID, C), F32R)
    gbe_d = {}
    for nm, n_ct in (("q", CT_C), ("k", CT_C), ("v", CT_C), ("p", CT_C),
                     ("f1", CT_H), ("f2", CT_C)):
        gbe_d[nm] = (dt_in(f"{nm}_gp", (P, n_ct), F32),
                     dt_in(f"{nm}_bp", (P, n_ct), F32))
    out_d = nc.dram_tensor("outT", [C, S], F32, kind="ExternalOutput").ap()

    with tile.TileContext(nc) as tc, ExitStack() as es:
        constp = es.enter_context(tc.tile_pool(name="const", bufs=1))
        gbep = es.enter_context(tc.tile_pool(name="gbep", bufs=1))
        statp = es.enter_context(tc.tile_pool(name="stats", bufs=1))
        scrp = es.enter_context(tc.tile_pool(name="scratch", bufs=2))
        bnp = es.enter_context(tc.tile_pool(name="bnconst", bufs=1))
        dramp = es.enter_context(tc.tile_pool(name="dram", bufs=2,
                                              space="DRAM"))

        eps_t = constp.tile([P, 1], F32, tag="eps")
        nc.vector.memset(eps_t[:], EPS)
        ident = constp.tile([P, P], BF16, tag="ident")
        make_identity(nc, ident[:])

        gbe_sb = {}
        for nm in gbe_d:
            n_ct = CT_H if nm == "f1" else CT_C
            gt = gbep.tile([P, n_ct], F32, tag=f"g_{nm}", name="gt")
            bt = gbep.tile([P, n_ct], F32, tag=f"b_{nm}", name="bt")
            nc.sync.dma_start(out=gt[:], in_=gbe_d[nm][0])
            nc.sync.dma_start(out=bt[:], in_=gbe_d[nm][1])
            gbe_sb[nm] = (gt, bt)

        # ---------------- shared helpers ----------------
        def drain(ps_ap, h_ap, sum_ap, sq_ap, eng="dve"):
            """PSUM -> SBUF h with per-partition sum (DVE or ACT); square
            pass (ACT) for sumsq."""
            if eng == "act":
                nc.scalar.activation(h_ap, ps_ap, AF.Copy,
                                     accum_out=sum_ap)
            else:
                nc.vector.tensor_copy(h_ap, ps_ap)
                nc.vector.tensor_reduce(sum_ap, h_ap,
                                        axis=mybir.AxisListType.X, op=AX.add)
            w = h_ap.shape[-1]
            scr = scrp.tile([P, SLAB], F32, tag="scr", name="scr")
            nc.scalar.activation(scr[:, 0:w], h_ap, AF.Square,
                                 accum_out=sq_ap)

        def exchange_stats(arin_sb, width, stage):
            """AllGather [P,width] from 8 cores + local deterministic sum
            (or AllReduce when KERNEL_AG=0)."""
            ain = dramp.tile([P, width], F32, tag="arin", name="ain")
            nc.sync.dma_start(out=ain[:], in_=arin_sb)
            if NO_COLL or not USE_AG:
                aout1 = dramp.tile([P, width], F32, tag="arout1",
                                   addr_space="Shared", name="aout1")
                if NO_COLL:
                    nc.sync.dma_start(out=aout1[:], in_=ain[:])
                else:
                    nc.gpsimd.collective_compute(
                        "AllReduce", AX.add, replica_groups=RG,
                        ins=[ain.opt()], outs=[aout1.opt()])
                g = statp.tile([P, width], F32, tag=f"gst{stage}", name="g")
                nc.sync.dma_start(out=g[:], in_=aout1[:])
                return g
            aout = dramp.tile([NCORES, P, width], F32, tag="arout",
                              addr_space="Shared", name="aout")
            nc.gpsimd.collective_compute(
                "AllGather", AX.bypass, replica_groups=RG,
                ins=[ain.opt()], outs=[aout.opt()])
            g8 = statp.tile([P, NCORES * width], F32, tag=f"g8{stage}",
                            name="g8")
            for r in range(NCORES):
                nc.sync.dma_start(out=g8[:, r * width:(r + 1) * width],
                                  in_=aout[r])
            g = statp.tile([P, width], F32, tag=f"gst{stage}", name="g")
            nc.vector.tensor_tensor(out=g[:], in0=g8[:, 0:width],
                                    in1=g8[:, width:2 * width], op=AX.add)
            for r in range(2, NCORES):
                nc.vector.tensor_tensor(
                    out=g[:], in0=g[:],
                    in1=g8[:, r * width:(r + 1) * width], op=AX.add)
            return g

        def bn_affine(gstats, n_ct, g_sl, b_sl, stage):
            """gstats [128, 2*n_ct] = [sums | sumsqs] (global).
            Returns (a_sc, c_sc) [128, 4*n_ct]: per-t-scaled affine."""
            mean = bnp.tile([P, n_ct], F32, tag="mean", name="mean")
            var = bnp.tile([P, n_ct], F32, tag="var", name="var")
            tmpb = bnp.tile([P, n_ct], F32, tag="btmp", name="tmpb")
            nc.vector.tensor_scalar(mean[:], gstats[:, 0:n_ct],
                                    1.0 / S_TOT, None, AX.mult)
            nc.vector.tensor_scalar(var[:], gstats[:, n_ct:2 * n_ct],
                                    1.0 / S_TOT, None, AX.mult)
            nc.vector.tensor_mul(tmpb[:], mean[:], mean[:])
            nc.vector.tensor_tensor(out=var[:], in0=var[:], in1=tmpb[:],
                                    op=AX.subtract)
            nc.scalar.activation(var[:], var[:], AF.Sqrt, bias=eps_t[:])
            nc.vector.reciprocal(var[:], var[:])
            a0 = bnp.tile([P, n_ct], F32, tag="a0", name="a0")
            c0 = bnp.tile([P, n_ct], F32, tag="c0", name="c0")
            nc.vector.tensor_mul(a0[:], var[:], g_sl)
            nc.vector.tensor_mul(tmpb[:], mean[:], a0[:])
            nc.vector.tensor_tensor(out=c0[:], in0=b_sl, in1=tmpb[:],
                                    op=AX.subtract)
            a_sc = bnp.tile([P, 4 * n_ct], F32, tag=f"asc{stage}",
                            name="a_sc")
            c_sc = bnp.tile([P, 4 * n_ct], F32, tag=f"csc{stage}",
                            name="c_sc")
            for t in range(T):
                s = float(2.0 ** (t - 1))
                nc.vector.tensor_scalar(a_sc[:, t * n_ct:(t + 1) * n_ct],
                                        a0[:], s, None, AX.mult)
                nc.vector.tensor_scalar(c_sc[:, t * n_ct:(t + 1) * n_ct],
                                        c0[:], s, None, AX.mult)
            return a_sc, c_sc

        def lif_step(m_ap, h_slab, t, ct, n_ct, asc_csc, writer):
            """One LIF step on [128, SLAB] with scaled membrane (x 2^t):
            fused update+reset-of-previous (1 DVE op), then spike writer."""
            thr = float(2.0 ** t)
            if asc_csc is None:
                sa, sc = float(2.0 ** (t - 1)), 0.0
            else:
                a_sc, c_sc = asc_csc
                sa = a_sc[:, t * n_ct + ct:t * n_ct + ct + 1]
                sc = c_sc[:, t * n_ct + ct:t * n_ct + ct + 1]
            if t == 0:
                if asc_csc is None:
                    nc.vector.tensor_scalar(m_ap, h_slab, sa, None, AX.mult)
                else:
                    nc.vector.tensor_scalar(m_ap, h_slab, sa, sc,
                                            AX.mult, AX.add)
            elif FUSED_LIF:
                nc.vector._custom_dve(LIF_UPD, out=m_ap, in0=h_slab,
                                      in1=m_ap, s0=sa, s1=sc,
                                      imm2=float(2.0 ** (t - 1)))
            else:
                nc.vector._custom_dve(TENSOR_MASK, out=m_ap, in0=m_ap,
                                      in1=m_ap, s0=float(2.0 ** (t - 1)),
                                      imm2=0.0)
                nc.vector.affine_then_add(m_ap, h_slab, m_ap, sa, sc)
            writer(m_ap, thr)

        hp_d = dramp.tile([CT_C, P, S], F32, tag="hpstage", name="hp_d")
        xr_d = dramp.tile([CT_C, P, S], F32, tag="xrstage", name="xr_d")
        hf2_d = dramp.tile([CT_C, P, S], F32, tag="hf2stage", name="hf2_d")

        # =======================================================
        # Phase 1: qkv matmuls + batched stats exchange
        # =======================================================
        with ExitStack() as es_h1:   # h_qkv: lives through phase 2
            hqp = es_h1.enter_context(tc.tile_pool(name="hqkv", bufs=12))
            psA_cm = tc.tile_pool(name="psA", bufs=4, space="PSUM")
            psA = psA_cm.__enter__()  # lives through phase 3 (LIFO ok)

            names3 = ("q", "k", "v")
            sumst = statp.tile([P, 3 * CT_C * T], F32, tag="sumst")
            sqst = statp.tile([P, 3 * CT_C * T], F32, tag="sqst")
            h_qkv = {}
            with ExitStack() as es1:
                xp = es1.enter_context(tc.tile_pool(name="xtiles", bufs=4))
                wqp = es1.enter_context(tc.tile_pool(name="wqkv", bufs=6))
                xh = []
                for ct in range(CT_C):
                    th = xp.tile([P, S], F32R, tag="xs", name="xh")
                    xh.append(th)
                wts_all = {}
                for kc in range(CT_C):
                    w2 = wqp.tile([P, C], F32R, tag="wq", name="wq")
                    nc.sync.dma_start(
                        out=w2[:], in_=w_lin["q"][kc * P:(kc + 1) * P, :])
                    wts_all.setdefault("q", []).append(w2)
                for ts in range(T):
                    for ct in range(CT_C):
                        nc.sync.dma_start(
                            out=xh[ct][:, ts * SLAB:(ts + 1) * SLAB],
                            in_=xt[ct * P:(ct + 1) * P,
                                   ts * SLAB:(ts + 1) * SLAB])
                for kc in range(CT_C):
                    nc.vector.tensor_reduce(xsum[:, kc:kc + 1],
                                            xh[kc][:].bitcast(F32),
                                            axis=mybir.AxisListType.X,
                                            op=AX.add)
                for li, nm in enumerate(names3):
                    if nm in wts_all:
                        wts = wts_all[nm]
                    else:
                        wts = []
                        for kc in range(CT_C):
                            w2 = wqp.tile([P, C], F32R, tag="wq", name="wq")
                            nc.sync.dma_start(
                                out=w2[:],
                                in_=w_lin[nm][kc * P:(kc + 1) * P, :])
                            wts.append(w2)
                    tiles = []
                    for ct in range(CT_C):
                        h_t = hqp.tile([P, S], F32, tag="h", name=f"h_{nm}")
                        tiles.append(h_t)
                        for ts in range(T):
                            ps = psA.tile([P, SLAB], F32, tag="ps",
                                          name="ps")
                            for kc in range(CT_C):
                                nc.tensor.matmul(
                                    ps[:], wts[kc][:, ct * P:(ct + 1) * P],
                                    xh[kc][:, ts * SLAB:(ts + 1) * SLAB],
                                    start=(kc == 0), stop=(kc == CT_C - 1))
                            col = li * CT_C * T + ct * T + ts
                            drain(ps[:], h_t[:, ts * SLAB:(ts + 1) * SLAB],
                                  sumst[:, col:col + 1],
                                  sqst[:, col:col + 1])
                    h_qkv[nm] = tiles

            arin = statp.tile([P, 2 * 3 * CT_C], F32, tag="arin_sb")
            nc.vector.tensor_reduce(
                arin[:, 0:3 * CT_C],
                sumst[:].rearrange("p (c t) -> p c t", t=T),
                axis=mybir.AxisListType.X, op=AX.add)
            nc.vector.tensor_reduce(
                arin[:, 3 * CT_C:6 * CT_C],
                sqst[:].rearrange("p (c t) -> p c t", t=T),
                axis=mybir.AxisListType.X, op=AX.add)
            gst = exchange_stats(arin[:], 2 * 3 * CT_C, "qkv")
            abc = {}
            for li, nm in enumerate(names3):
                g_t, b_t = gbe_sb[nm]
                gsl = statp.tile([P, 2 * CT_C], F32, tag=f"gsl{nm}",
                                 name="gsl")
                nc.vector.tensor_copy(
                    gsl[:, 0:CT_C], gst[:, li * CT_C:(li + 1) * CT_C])
                nc.vector.tensor_copy(
                    gsl[:, CT_C:2 * CT_C],
                    gst[:, (3 + li) * CT_C:(4 + li) * CT_C])
                abc[nm] = bn_affine(gsl, CT_C, g_t[:], b_t[:], nm)

            # =======================================================
            # Phase 2: per-ts pipeline qkv-LIF -> attention -> o-LIF ->
            # p matmul (h_p staged to DRAM)
            # =======================================================
            sumst_p = statp.tile([P, CT_C * T], F32, tag="sumstp")
            sqst_p = statp.tile([P, CT_C * T], F32, tag="sqstp")
            arin_p = statp.tile([P, 2 * CT_C], F32, tag="arinp")
            if PHASES < 2:
                raise _PhaseStop
            with ExitStack() as es2:
                wpp = es2.enter_context(tc.tile_pool(name="wp", bufs=1))
                mqkv = es2.enter_context(tc.tile_pool(name="mqkv", bufs=1))
                op_pool = es2.enter_context(tc.tile_pool(name="otile", bufs=8))
                hpp = es2.enter_context(tc.tile_pool(name="hp", bufs=6))
                atp = es2.enter_context(tc.tile_pool(name="attn", bufs=6))
                kvp = es2.enter_context(tc.tile_pool(name="kvp", bufs=4))
                pst = es2.enter_context(tc.tile_pool(name="pst", bufs=1, space="PSUM"))
                pskv = es2.enter_context(tc.tile_pool(name="pskv", bufs=1, space="PSUM"))
                pso = es2.enter_context(tc.tile_pool(name="pso", bufs=1, space="PSUM"))

                wpt = []
                for kc in range(CT_C):
                    w2 = wpp.tile([P, C], F32R, tag=f"wp{kc}", name="wp")
                    nc.sync.dma_start(
                        out=w2[:], in_=w_lin["p"][kc * P:(kc + 1) * P, :])
                    wpt.append(w2)

                m_t = {}
                for nm in names3:
                    for ct in range(CT_C):
                        m_t[(nm, ct)] = mqkv.tile(
                            [P, SLAB], F32, tag=f"m{nm}{ct}", name="m_t")
                m_o = [mqkv.tile([P, SLAB], F32, tag=f"mo{ct}", name="m_o")
                       for ct in range(CT_C)]

                # spike APs: bf16 written into dead h slab bytes (in place)
                def spk_ap(nm, ct, ts, c0, c1):
                    base = 2 * SLAB * ts
                    return h_qkv[nm][ct][:].bitcast(BF16)[
                        :, base + c0:base + c1]

                def qkv_lif(ts):
                    for nm in names3:
                        for ct in range(CT_C):
                            h_t = h_qkv[nm][ct]

                            def wr(m_ap, thr, nm=nm, ct=ct, ts=ts):
                                nc.vector.tensor_scalar(
                                    spk_ap(nm, ct, ts, 0, SLAB), m_ap,
                                    thr, None, AX.is_gt)

                            lif_step(m_t[(nm, ct)][:],
                                     h_t[:, ts * SLAB:(ts + 1) * SLAB],
                                     ts, ct, CT_C, abc[nm], wr)

                def attn_olif_p(ts):
                    o_slabs = [op_pool.tile([P, SLAB], F32, tag="o",
                                            name="o_slab")
                               for _ in range(CT_C)]
                    for bi in range(BL):
                        sb0 = bi * N  # token offset within ts slab
                        kvTT = atp.tile([P, 4 * C], BF16, tag="kvT",
                                        name="kvTT")

                        def kv_cols(src, hf_tok, hd):
                            # ct block: [k-hf0 | k-hf1 | v-hf0 | v-hf1]
                            base = (4 * (hd // 2) * P + src * 2 * P
                                    + hf_tok * P + (hd % 2) * 64)
                            return slice(base, base + 64)

                        for ct in range(CT_C):
                            pt = pst.tile([P, 4 * P], BF16, tag="pt",
                                          name="pt")
                            for si, srcnm in enumerate(("k", "v")):
                                for hf in range(2):
                                    nc.tensor.transpose(
                                        pt[:, (2 * si + hf) * P:
                                           (2 * si + hf + 1) * P],
                                        spk_ap(srcnm, ct, ts, sb0 + hf * P,
                                               sb0 + (hf + 1) * P),
                                        ident[:])
                            nc.scalar.copy(
                                kvTT[:, 4 * ct * P:4 * (ct + 1) * P], pt[:])
                        for ct in range(CT_C):
                            po = pso.tile([P, N], F32, tag="po", name="po")
                            pkv = pskv.tile([P, 64], F32, tag="pkv",
                                            name="pkv")
                            for hf in range(2):
                                hd = ct * 2 + hf
                                prt = slice(hf * 64, (hf + 1) * 64)
                                nc.tensor.matmul(pkv[prt, :],
                                                 kvTT[:, kv_cols(0, 0, hd)],
                                                 kvTT[:, kv_cols(1, 0, hd)],
                                                 start=True, stop=False)
                                nc.tensor.matmul(pkv[prt, :],
                                                 kvTT[:, kv_cols(0, 1, hd)],
                                                 kvTT[:, kv_cols(1, 1, hd)],
                                                 start=False, stop=True)
                            kv_sb = kvp.tile([P, 64], BF16, tag="kv",
                                             name="kv_sb")
                            nc.scalar.mul(kv_sb[:], pkv[:], SCALE)
                            for hf in range(2):
                                prt = slice(hf * 64, (hf + 1) * 64)
                                nc.tensor.matmul(
                                    po[prt, :], kv_sb[prt, :],
                                    spk_ap("q", ct, ts, sb0, sb0 + N)[prt, :],
                                    start=True, stop=True)
                            nc.scalar.copy(
                                o_slabs[ct][:].bitcast(F32R)[:, sb0:sb0 + N],
                                po[:])
                    # o-LIF (no BN; spikes f32r in place) + p matmul
                    for ct in range(CT_C):
                        o_s = o_slabs[ct]

                        def wr_o(m_ap, thr, o_s=o_s):
                            nc.vector.tensor_scalar(o_s[:].bitcast(F32R),
                                                    m_ap, thr, None,
                                                    AX.is_gt)

                        lif_step(m_o[ct][:], o_s[:], ts, ct, CT_C, None,
                                 wr_o)
                    for ct in range(CT_C):
                        ps = psA.tile([P, SLAB], F32, tag="ps", name="ps")
                        for kc in range(CT_C):
                            nc.tensor.matmul(
                                ps[:], wpt[kc][:, ct * P:(ct + 1) * P],
                                o_slabs[kc][:].bitcast(F32R),
                                start=(kc == 0), stop=(kc == CT_C - 1))
                        hp_s = hpp.tile([P, SLAB], F32, tag="hps",
                                        name="hp_s")
                        col = ct * T + ts
                        drain(ps[:], hp_s[:], sumst_p[:, col:col + 1],
                              sqst_p[:, col:col + 1], eng="act")
                        nc.sync.dma_start(
                            out=hp_d[ct][:, ts * SLAB:(ts + 1) * SLAB],
                            in_=hp_s[:])

                for ts in range(T):
                    qkv_lif(ts)
                    if ts >= 1:
                        attn_olif_p(ts - 1)
                attn_olif_p(T - 1)
                nc.vector.tensor_reduce(
                    osum[:], osum_st[:].rearrange("p (c t) -> p c t", t=T),
                    axis=mybir.AxisListType.X, op=AX.add)
                ps_mp = psM.tile([P, CT_C], F32, tag="psm", name="ps_mp")
                for ct in range(CT_C):
                    for kc in range(CT_C):
                        nc.tensor.matmul(
                            ps_mp[:, ct:ct + 1],
                            wpt[kc][:, ct * P:(ct + 1) * P],
                            osum[:, kc:kc + 1],
                            start=(kc == 0), stop=(kc == CT_C - 1))
                nc.scalar.copy(arin_p[:, 0:CT_C], ps_mp[:])

        # h_qkv + phase-2 pools closed; psA still open
        if PHASES < 3:
            raise _PhaseStop
        nc.vector.tensor_reduce(
            arin_p[:, 0:CT_C],
            sumst_p[:].rearrange("p (c t) -> p c t", t=T),
            axis=mybir.AxisListType.X, op=AX.add)
        nc.vector.tensor_reduce(
            arin_p[:, CT_C:2 * CT_C],
            sqst_p[:].rearrange("p (c t) -> p c t", t=T),
            axis=mybir.AxisListType.X, op=AX.add)
        gst_p = exchange_stats(arin_p[:], 2 * CT_C, "p")
        g_t, b_t = gbe_sb["p"]
        abc_p = bn_affine(gst_p, CT_C, g_t[:], b_t[:], "p")

        # =======================================================
        # Phase 3: p-LIF (fused spike+residual -> xr) + f1 matmul,
        # per-ts streaming of x and h_p slabs; xr staged to DRAM
        # =======================================================
        sumst_f = statp.tile([P, CT_H * T], F32, tag="sumstf")
        sqst_f = statp.tile([P, CT_H * T], F32, tag="sqstf")
        arin_f = statp.tile([P, 2 * CT_H], F32, tag="arinf")
        with ExitStack() as es_hf1:  # h_f1: lives through phase 4
            hf1p = es_hf1.enter_context(tc.tile_pool(name="hf1", bufs=16))
            h_f1 = [hf1p.tile([P, S], F32, tag="hf1", name="h_f1")
                    for _ in range(CT_H)]
            with ExitStack() as es3:
                wf1p = es3.enter_context(tc.tile_pool(name="wf1", bufs=1))
                mpp = es3.enter_context(tc.tile_pool(name="mp", bufs=1))
                xsp = es3.enter_context(tc.tile_pool(name="xslab", bufs=6))
                hpsp = es3.enter_context(tc.tile_pool(name="hpslab", bufs=5))

                wf1t = []
                for kc in range(CT_C):
                    w2 = wf1p.tile([P, HID], F32R, tag=f"wf1{kc}",
                                   name="wf1")
                    nc.sync.dma_start(out=w2[:],
                                      in_=w_f1[kc * P:(kc + 1) * P, :])
                    wf1t.append(w2)
                m_p = [mpp.tile([P, SLAB], F32, tag=f"mp{ct}", name="m_p")
                       for ct in range(CT_C)]

                xs_all = {}

                def p_lif(ts):
                    xs_t, hp_t = [], []
                    for ct in range(CT_C):
                        xs = xsp.tile([P, SLAB], F32R, tag="xsl", name="xs")
                        nc.sync.dma_start(
                            out=xs[:],
                            in_=xt[ct * P:(ct + 1) * P,
                                   ts * SLAB:(ts + 1) * SLAB])
                        xs_t.append(xs)
                        hp_s = hpsp.tile([P, SLAB], F32, tag="hpl",
                                         name="hp_s")
                        nc.sync.dma_start(
                            out=hp_s[:],
                            in_=hp_d[ct][:, ts * SLAB:(ts + 1) * SLAB])
                        hp_t.append(hp_s)
                    xs_all[ts] = xs_t
                    for ct in range(CT_C):
                        xr_r = xs_t[ct][:]

                        def wr_p(m_ap, thr, xr_r=xr_r):
                            if USE_STT:
                                nc.vector.scalar_tensor_tensor(
                                    out=xr_r, in0=m_ap, scalar=thr,
                                    in1=xr_r, op0=AX.is_gt, op1=AX.add)
                            else:
                                tmp = scrp.tile([P, SLAB], F32, tag="stmp",
                                                name="tmp")
                                nc.vector.tensor_scalar(
                                    tmp[:], m_ap, thr, None, AX.is_gt)
                                nc.vector.tensor_tensor(
                                    out=xr_r, in0=xr_r, in1=tmp[:],
                                    op=AX.add)

                        lif_step(m_p[ct][:], hp_t[ct][:], ts, ct, CT_C,
                                 abc_p, wr_p)

                def f1_mm(ts):
                    xs_t = xs_all.pop(ts)
                    for hct in range(CT_H):
                        ps = psA.tile([P, SLAB], F32, tag="ps", name="ps")
                        for kc in range(CT_C):
                            nc.tensor.matmul(
                                ps[:], wf1t[kc][:, hct * P:(hct + 1) * P],
                                xs_t[kc][:],
                                start=(kc == 0), stop=(kc == CT_C - 1))
                        col = hct * T + ts
                        drain(ps[:], h_f1[hct][:, ts * SLAB:(ts + 1) * SLAB],
                              sumst_f[:, col:col + 1],
                              sqst_f[:, col:col + 1], eng="act")
                    for ct in range(CT_C):
                        nc.sync.dma_start(
                            out=xr_d[ct][:, ts * SLAB:(ts + 1) * SLAB],
                            in_=xs_t[ct][:].bitcast(F32))

                for ts in range(T):
                    p_lif(ts)
                    if ts >= 1:
                        f1_mm(ts - 1)
                f1_mm(T - 1)
                nc.vector.tensor_reduce(
                    xrsum[:],
                    xrsum_st[:].rearrange("p (c t) -> p c t", t=T),
                    axis=mybir.AxisListType.X, op=AX.add)
                ps_mf = psM.tile([P, CT_H], F32, tag="psm", name="ps_mf")
                for hct in range(CT_H):
                    for kc in range(CT_C):
                        nc.tensor.matmul(
                            ps_mf[:, hct:hct + 1],
                            wf1t[kc][:, hct * P:(hct + 1) * P],
                            xrsum[:, kc:kc + 1],
                            start=(kc == 0), stop=(kc == CT_C - 1))
                nc.scalar.copy(arin_f_sums[:], ps_mf[:])

            psA_cm.__exit__(None, None, None)

            nc.vector.tensor_reduce(
                arin_f[:, 0:CT_H],
                sumst_f[:].rearrange("p (c t) -> p c t", t=T),
                axis=mybir.AxisListType.X, op=AX.add)
            nc.vector.tensor_reduce(
                arin_f[:, CT_H:2 * CT_H],
                sqst_f[:].rearrange("p (c t) -> p c t", t=T),
                axis=mybir.AxisListType.X, op=AX.add)
            gst_f = exchange_stats(arin_f[:], 2 * CT_H, "f1")
            g_t, b_t = gbe_sb["f1"]
            abc_f1 = bn_affine(gst_f, CT_H, g_t[:], b_t[:], "f1")

            # =======================================================
            # Phase 4: f1-LIF tile-wise (spikes f32r in place) + f2
            # partial accumulation: ts01 behind LIF, then ts23 sweep;
            # h_f2 slabs staged to DRAM
            # =======================================================
            sumst_2 = statp.tile([P, CT_C * 2], F32, tag="sumst2")
            sqst_2 = statp.tile([P, CT_C * 2], F32, tag="sqst2")
            if PHASES < 4:
                raise _PhaseStop
            with ExitStack() as es4:
                wf2p = es4.enter_context(tc.tile_pool(name="wf2", bufs=16))
                hf2p = es4.enter_context(tc.tile_pool(name="hf2", bufs=4))
                mf1p = es4.enter_context(tc.tile_pool(name="mf1", bufs=8))
                psf2 = es4.enter_context(
                    tc.tile_pool(name="psf2", bufs=8, space="PSUM"))

                wf2t = []
                for kc in range(CT_H):
                    w2 = wf2p.tile([P, C], F32R, tag="wf2", name="wf2")
                    nc.sync.dma_start(out=w2[:],
                                      in_=w_f2[kc * P:(kc + 1) * P, :])
                    wf2t.append(w2)
                sumst_2 = statp.tile([P, CT_C * T], F32, tag="sumst2")
                sqst_2 = statp.tile([P, CT_C * T], F32, tag="sqst2")

                ps2_a = [psf2.tile([P, SLAB], F32, tag="ps2",
                                   name="ps2_a") for _ in range(2 * CT_C)]
                # f1-LIF with per-ts membranes; spikes go to a fresh F32R
                # tile that ring-reuses h_f1[kc]'s buffer (tag match)
                spk_f1 = []
                for kc in range(CT_H):
                    on_pool = USE_POOL and (kc % 4 == 3)
                    m_ts = []
                    for t in range(T):
                        m = mf1p.tile([P, SLAB], F32, tag="mf1", name="m")
                        thr = float(2.0 ** t)
                        a_sc, c_sc = abc_f1
                        sa = a_sc[:, t * CT_H + kc:t * CT_H + kc + 1]
                        sc = c_sc[:, t * CT_H + kc:t * CT_H + kc + 1]
                        h_sl = h_f1[kc][:, t * SLAB:(t + 1) * SLAB]
                        if on_pool:
                            if t == 0:
                                nc.gpsimd.tensor_scalar(m[:], h_sl, sa, sc,
                                                        AX.mult, AX.add)
                            else:
                                mprev = m_ts[t - 1][:]
                                nc.gpsimd.scalar_tensor_tensor(
                                    out=mprev, in0=mprev,
                                    scalar=float(2.0 ** (t - 1)),
                                    in1=mprev, op0=AX.is_le, op1=AX.mult)
                                nc.gpsimd.scalar_tensor_tensor(
                                    out=m[:], in0=h_sl, scalar=sa,
                                    in1=mprev, op0=AX.mult, op1=AX.add)
                                nc.gpsimd.tensor_scalar(
                                    m[:], m[:], sc, None, AX.add)
                        elif t == 0:
                            nc.vector.tensor_scalar(m[:], h_sl, sa, sc,
                                                    AX.mult, AX.add)
                        elif FUSED_LIF:
                            nc.vector._custom_dve(
                                LIF_UPD, out=m[:], in0=h_sl,
                                in1=m_ts[t - 1][:], s0=sa, s1=sc,
                                imm2=float(2.0 ** (t - 1)))
                        else:
                            nc.vector._custom_dve(
                                TENSOR_MASK, out=m[:], in0=m_ts[t - 1][:],
                                in1=m_ts[t - 1][:],
                                s0=float(2.0 ** (t - 1)), imm2=0.0)
                            nc.vector.affine_then_add(m[:], h_sl, m[:],
                                                      sa, sc)
                        m_ts.append(m)
                    s_t = hf1p.tile([P, S], F32R, tag="hf1", name="spk_f1")
                    spk_f1.append(s_t)
                    eng_s = nc.gpsimd if on_pool else nc.vector
                    for t in range(T):
                        eng_s.tensor_scalar(
                            s_t[:, t * SLAB:(t + 1) * SLAB], m_ts[t][:],
                            float(2.0 ** t), None, AX.is_gt)
                    # f2 partial accumulation for ts0+ts1
                    if INTERLEAVE_F2:
                        for ct in range(CT_C):
                            for tsl in range(2):
                                nc.tensor.matmul(
                                    ps2_a[ct * 2 + tsl][:],
                                    wf2t[kc][:, ct * P:(ct + 1) * P],
                                    s_t[:, tsl * SLAB:(tsl + 1) * SLAB],
                                    start=(kc == 0), stop=(kc == CT_H - 1),
                                    skip_group_check=True)
                if not INTERLEAVE_F2:
                    for ct in range(CT_C):
                        for tsl in range(2):
                            for kc in range(CT_H):
                                nc.tensor.matmul(
                                    ps2_a[ct * 2 + tsl][:],
                                    wf2t[kc][:, ct * P:(ct + 1) * P],
                                    spk_f1[kc][:, tsl * SLAB:
                                               (tsl + 1) * SLAB],
                                    start=(kc == 0), stop=(kc == CT_H - 1))
                for ct in range(CT_C):
                    for tsl in range(2):
                        col = ct * T + tsl
                        h2s = hf2p.tile([P, SLAB], F32, tag="h2s",
                                        name="h2s")
                        drain(ps2_a[ct * 2 + tsl][:], h2s[:],
                              sumst_2[:, col:col + 1],
                              sqst_2[:, col:col + 1], eng="act")
                        nc.sync.dma_start(
                            out=hf2_d[ct][:, tsl * SLAB:(tsl + 1) * SLAB],
                            in_=h2s[:])
                ps2_b = [psf2.tile([P, SLAB], F32, tag="ps2", name="ps2_b")
                         for _ in range(2 * CT_C)]
                for kc in range(CT_H):
                    for ct in range(CT_C):
                        for tsl in range(2, 4):
                            nc.tensor.matmul(
                                ps2_b[ct * 2 + tsl - 2][:],
                                wf2t[kc][:, ct * P:(ct + 1) * P],
                                spk_f1[kc][:, tsl * SLAB:(tsl + 1) * SLAB],
                                start=(kc == 0), stop=(kc == CT_H - 1),
                                skip_group_check=True)
                for ct in range(CT_C):
                    for tsl in range(2, 4):
                        col = ct * T + tsl
                        h2s = hf2p.tile([P, SLAB], F32, tag="h2s",
                                        name="h2s")
                        drain(ps2_b[ct * 2 + tsl - 2][:], h2s[:],
                              sumst_2[:, col:col + 1],
                              sqst_2[:, col:col + 1], eng="act")
                        nc.sync.dma_start(
                            out=hf2_d[ct][:, tsl * SLAB:(tsl + 1) * SLAB],
                            in_=h2s[:])

        # h_f1 + phase-4 pools closed
        if PHASES < 5:
            raise _PhaseStop
        arin_2 = statp.tile([P, 2 * CT_C], F32, tag="arin2")
        nc.vector.tensor_reduce(
            arin_2[:, 0:CT_C],
            sumst_2[:].rearrange("p (c t) -> p c t", t=T),
            axis=mybir.AxisListType.X, op=AX.add)
        nc.vector.tensor_reduce(
            arin_2[:, CT_C:2 * CT_C],
            sqst_2[:].rearrange("p (c t) -> p c t", t=T),
            axis=mybir.AxisListType.X, op=AX.add)
        gst_2 = exchange_stats(arin_2[:], 2 * CT_C, "f2")
        g_t, b_t = gbe_sb["f2"]
        abc_f2 = bn_affine(gst_2, CT_C, g_t[:], b_t[:], "f2")

        # =======================================================
        # Phase 5: f2-LIF + residual over streamed xr; output in place
        # =======================================================
        with ExitStack() as es5:
            xrp = es5.enter_context(tc.tile_pool(name="xrp", bufs=2))
            h2rp = es5.enter_context(tc.tile_pool(name="h2r", bufs=2))
            mf2p = es5.enter_context(tc.tile_pool(name="mf2", bufs=2))
            for ct in range(CT_C):
                xr_t = xrp.tile([P, S], F32, tag="xr", name="xr_t")
                nc.sync.dma_start(out=xr_t[:], in_=xr_d[ct])
                h2_t = h2rp.tile([P, S], F32, tag="h2r", name="h2_t")
                nc.sync.dma_start(out=h2_t[:], in_=hf2_d[ct])
                m = mf2p.tile([P, SLAB], F32, tag="mf2", name="m")
                for ts in range(T):
                    def wr_2(m_ap, thr, xr_t=xr_t, ts=ts):
                        sl = xr_t[:, ts * SLAB:(ts + 1) * SLAB]
                        if USE_STT:
                            nc.vector.scalar_tensor_tensor(
                                out=sl, in0=m_ap, scalar=thr, in1=sl,
                                op0=AX.is_gt, op1=AX.add)
                        else:
                            tmp = scrp.tile([P, SLAB], F32, tag="stmp",
                                            name="tmp")
                            nc.vector.tensor_scalar(
                                tmp[:], m_ap, thr, None, AX.is_gt)
                            nc.vector.tensor_tensor(
                                out=sl, in0=sl, in1=tmp[:], op=AX.add)

                    lif_step(m[:], h2_t[:, ts * SLAB:(ts + 1) * SLAB],
                             ts, ct, CT_C, abc_f2, wr_2)
                    nc.sync.dma_start(
                        out=out_d[ct * P:(ct + 1) * P,
                                  ts * SLAB:(ts + 1) * SLAB],
                        in_=xr_t[:, ts * SLAB:(ts + 1) * SLAB])

    nc.compile()
    return nc


def _get_nc():
    if "nc" not in _CACHE:
        _CACHE["nc"] = _build()
    return _CACHE["nc"]


def _make_in_maps(inputs):
    x = np.asarray(inputs["x"], np.float32)
    base = {}
    for nm in ("q", "k", "v", "p"):
        base[f"w_{nm}"] = _round_mant(np.asarray(inputs[f"{nm}_W"],
                                                 np.float32))
    base["w_f1"] = _round_mant(np.asarray(inputs["f1_W"], np.float32))
    base["w_f2"] = _round_mant(np.asarray(inputs["f2_W"], np.float32))
    for nm, n_ct in (("q", CT_C), ("k", CT_C), ("v", CT_C), ("p", CT_C),
                     ("f1", CT_H), ("f2", CT_C)):
        base[f"{nm}_gp"] = _pack_ch(inputs[f"{nm}_g"], n_ct)
        base[f"{nm}_bp"] = _pack_ch(inputs[f"{nm}_be"], n_ct)
    in_maps = []
    for c in range(NCORES):
        xs = x[:, c * BL:(c + 1) * BL].reshape(S, C)
        m = dict(base)
        m["xt"] = _round_mant(np.ascontiguousarray(xs.T))
        in_maps.append(m)
    return in_maps


def kernel(**inputs):
    in_maps = _make_in_maps(inputs)
    nc = _get_nc()
    res = bass_utils.run_bass_kernel_spmd(nc, in_maps,
                                          core_ids=list(range(NCORES)))
    _CACHE["last_results"] = res

    out = np.empty((T, B, N, C), np.float32)
    for c in range(NCORES):
        oc = np.asarray(res.results[c]["outT"])   # [C, S]
        out[:, c * BL:(c + 1) * BL] = oc.T.reshape(T, BL, N, C)
    return out
